# revision 8
# baseline (speedup 1.0000x reference)
"""CNAttnProcessor cross-attention kernel for 8 Trainium2 NeuronCores.

Data-parallel over batch: 16 batches -> 2 per core. Per batch computes
  x = hidden[b] viewed [C=320, HW=4096]  (feature-major = "s-major")
  q^T = Wq_pad^T-arranged @ x           [512pad, 4096]
  k^T, v from enc[b,:77]                 (enc transposed on PE)
  scores^T_h = k_h^T.T @ q_h^T           [77, t]
  E = exp(SCALE * scores)                (unnormalized probs)
  attn_un^T = [v_h | 1col].T @ E_h       -> packed [512pad, t], den rows at 64h+63
  normalize via R = sel^T @ recip(den8)  (replication matmul, K=8)
  final^T = Wo_pad^T @ attn_norm^T + bo + residual
"""

import numpy as np

import concourse.bass as bass
import concourse.mybir as mybir
import concourse.tile as tile
from concourse import bacc, bass_utils

F32 = mybir.dt.float32
HEADS = 8
DH = 40          # real head dim
DP = 64          # padded head dim
C = 320
CP = 512         # padded channels (8 * 64)
SK = 77          # encoder tokens kept
DC = 768
HW = 4096
BPC = 2          # batches per core
NCORES = 8
TCH = 512        # token chunk
NT = HW // TCH   # 8 chunks
SCALE = float(1.0 / np.sqrt(np.float32(DH)))

_CACHE = {}


def _build_nc():
    nc = bacc.Bacc("TRN2", target_bir_lowering=False, debug=False, num_devices=1)

    xin = nc.dram_tensor("xin", [BPC, C, HW], F32, kind="ExternalInput").ap()
    enc = nc.dram_tensor("enc", [BPC, SK, DC], F32, kind="ExternalInput").ap()
    wq = nc.dram_tensor("wq", [128, 3, CP], F32, kind="ExternalInput").ap()
    wk = nc.dram_tensor("wk", [128, 6, CP], F32, kind="ExternalInput").ap()
    wv = nc.dram_tensor("wv", [128, 6, CP], F32, kind="ExternalInput").ap()
    wo = nc.dram_tensor("wo", [128, 4, C], F32, kind="ExternalInput").ap()
    sel = nc.dram_tensor("sel", [8, 4, 128], F32, kind="ExternalInput").ap()
    bo = nc.dram_tensor("bo", [C, 1], F32, kind="ExternalInput").ap()
    ident = nc.dram_tensor("ident", [128, 128], F32, kind="ExternalInput").ap()
    xout = nc.dram_tensor("xout", [BPC, C, HW], F32, kind="ExternalOutput").ap()

    MW = [128, 128, 64]          # C chunks (320)
    MS = [0, 128, 256]
    KWQ = [128, 128, 64]         # contraction chunks for C=320

    with tile.TileContext(nc) as tc:
        with (
            tc.tile_pool(name="const", bufs=1) as const,
            tc.tile_pool(name="xpool", bufs=1) as xpool,
            tc.tile_pool(name="small", bufs=2) as small,
            tc.tile_pool(name="qpool", bufs=2) as qpool,
            tc.tile_pool(name="epool", bufs=2) as epool,
            tc.tile_pool(name="apool", bufs=2) as apool,
            tc.tile_pool(name="opool", bufs=2) as opool,
            tc.tile_pool(name="dpool", bufs=3) as dpool,
            tc.tile_pool(name="ps", bufs=2, space="PSUM") as ps,
        ):
            # ---- constants (once) ----
            wq_sb = const.tile([128, 3, CP], F32)
            nc.sync.dma_start(out=wq_sb, in_=wq)
            wk_sb = const.tile([128, 6, CP], F32)
            nc.sync.dma_start(out=wk_sb, in_=wk)
            wv_sb = const.tile([128, 6, CP], F32)
            nc.sync.dma_start(out=wv_sb, in_=wv)
            wo_sb = const.tile([128, 4, C], F32)
            nc.sync.dma_start(out=wo_sb, in_=wo)
            sel_sb = const.tile([8, 4, 128], F32)
            nc.sync.dma_start(out=sel_sb, in_=sel)
            bo_sb = const.tile([128, 3, 1], F32)
            for m in range(3):
                nc.sync.dma_start(out=bo_sb[: MW[m], m, :], in_=bo[MS[m] : MS[m] + MW[m], :])
            id_sb = const.tile([128, 128], F32)
            nc.sync.dma_start(out=id_sb, in_=ident)

            for b in range(BPC):
                # ---- load x^T (feature-major; also the residual) ----
                xt = []
                for m in range(3):
                    t = xpool.tile([128, HW], F32, tag=f"xt{m}")
                    nc.sync.dma_start(out=t[: MW[m], :], in_=xin[b, MS[m] : MS[m] + MW[m], :])
                    xt.append(t)

                # ---- enc -> enc^T via PE transpose ----
                enc_sb = small.tile([SK, DC], F32, tag="enc")
                nc.sync.dma_start(out=enc_sb, in_=enc[b])
                encT = small.tile([128, 6, SK], F32, tag="encT")
                for j in range(6):
                    ps_tr = ps.tile([128, 4, TCH], F32, tag="ps", name="ps_tr")[:, 0, :SK]
                    nc.tensor.transpose(
                        ps_tr[:, :SK], enc_sb[:, j * 128 : (j + 1) * 128], id_sb[:SK, :SK]
                    )
                    nc.any.tensor_copy(out=encT[:, j, :], in_=ps_tr)

                # ---- k^T (padded [512, 77] as 4 tiles) ----
                kt = []
                for m in range(4):
                    kps = ps.tile([128, 4, TCH], F32, tag="ps", name="kps")[:, 0, :SK]
                    for kk in range(6):
                        nc.tensor.matmul(
                            kps,
                            lhsT=wk_sb[:, kk, m * 128 : (m + 1) * 128],
                            rhs=encT[:, kk, :],
                            start=(kk == 0),
                            stop=(kk == 5),
                        )
                    t = small.tile([128, SK], F32, tag=f"kt{m}")
                    nc.any.tensor_copy(out=t, in_=kps)
                    kt.append(t)

                # ---- v' [77, 8, 64] with ones column at d=63 ----
                vps = ps.tile([128, 4, TCH], F32, tag="ps", name="vps")[:SK, 0, :CP]
                for kk in range(6):
                    nc.tensor.matmul(
                        vps,
                        lhsT=encT[:, kk, :],
                        rhs=wv_sb[:, kk, :],
                        start=(kk == 0),
                        stop=(kk == 5),
                    )
                v_sb = small.tile([SK, HEADS, DP], F32, tag="v")
                nc.any.tensor_copy(out=v_sb, in_=vps.rearrange("p (h d) -> p h d", h=HEADS))
                nc.vector.memset(v_sb[:, :, 63:64], 1.0)

                # ---- main loop over token chunks ----
                for n in range(NT):
                    tsl = bass.ts(n, TCH)

                    # Q projection for this chunk -> q_ps [128, 4, 512]
                    q_ps = ps.tile([128, 4, TCH], F32, tag="ps")
                    for m in range(4):
                        for kk in range(3):
                            kw = KWQ[kk]
                            nc.tensor.matmul(
                                q_ps[:, m, :],
                                lhsT=wq_sb[:kw, kk, m * 128 : (m + 1) * 128],
                                rhs=xt[kk][:kw, tsl],
                                start=(kk == 0),
                                stop=(kk == 2),
                            )
                    q_sb = qpool.tile([128, 4, TCH], F32, tag="q")
                    nc.any.tensor_copy(out=q_sb, in_=q_ps)

                    # scores + exp, two groups of 4 heads
                    e_sb = epool.tile([SK, HEADS, TCH], F32, tag="E")
                    for g in range(2):
                        sc_ps = ps.tile([128, 4, TCH], F32, tag="ps")
                        for j in range(4):
                            h = 4 * g + j
                            c, eps = h // 2, h % 2
                            nc.tensor.matmul(
                                sc_ps[:SK, j, :],
                                lhsT=kt[c][64 * eps : 64 * eps + DH, :],
                                rhs=q_sb[64 * eps : 64 * eps + DH, c, :],
                                start=True,
                                stop=True,
                            )
                        nc.scalar.activation(
                            out=e_sb[:SK, 4 * g : 4 * g + 4, :],
                            in_=sc_ps[:SK, :, :],
                            func=mybir.ActivationFunctionType.Exp,
                            scale=SCALE,
                        )

                    # AV (unnormalized) -> av_ps [128, 4, 512]; den rows at 64e+63
                    av_ps = ps.tile([128, 4, TCH], F32, tag="ps")
                    for h in range(HEADS):
                        c, eps = h // 2, h % 2
                        nc.tensor.matmul(
                            av_ps[64 * eps : 64 * eps + DP, c, :],
                            lhsT=v_sb[:SK, h, :],
                            rhs=e_sb[:SK, h, :],
                            start=True,
                            stop=True,
                        )

                    # unnormalized attn to SBUF (mandatory PSUM drain)
                    attn_un = qpool.tile([128, 4, TCH], F32, tag="r")
                    nc.any.tensor_copy(out=attn_un, in_=av_ps)

                    # gather denominators -> den8 [8, 512]; recip
                    den8 = dpool.tile([8, TCH], F32, tag="den")
                    for c in range(4):
                        nc.sync.dma_start(
                            out=den8[c : c + 5 : 4, :],
                            in_=attn_un[63 : 63 + 65 : 64, c, :],
                        )
                    rec8 = dpool.tile([8, TCH], F32, tag="rec")
                    nc.vector.reciprocal(out=rec8, in_=den8)

                    # replication matmul: R[:, c, :] rows 64e+j = recip(den of head 2c+e)
                    r_ps = ps.tile([128, 4, TCH], F32, tag="ps")
                    for c in range(4):
                        nc.tensor.matmul(
                            r_ps[:, c, :], lhsT=sel_sb[:, c, :], rhs=rec8, start=True, stop=True
                        )

                    # normalize: attn = attn_un * R   (R read from PSUM)
                    attn = apool.tile([128, 4, TCH], F32, tag="attn")
                    nc.vector.tensor_tensor(
                        out=attn, in0=attn_un, in1=r_ps, op=mybir.AluOpType.mult
                    )

                    # Wo projection (s-major out) + bias + residual
                    fo_ps = ps.tile([128, 3, TCH], F32, tag="ps")
                    for m in range(3):
                        mw = MW[m]
                        for kk in range(4):
                            nc.tensor.matmul(
                                fo_ps[:mw, m, :],
                                lhsT=wo_sb[:, kk, MS[m] : MS[m] + mw],
                                rhs=attn[:, kk, :],
                                start=(kk == 0),
                                stop=(kk == 3),
                            )
                    o_sb = opool.tile([128, 3, TCH], F32, tag="o")
                    for m in range(3):
                        mw = MW[m]
                        nc.vector.scalar_tensor_tensor(
                            out=o_sb[:mw, m, :],
                            in0=fo_ps[:mw, m, :],
                            scalar=bo_sb[:mw, m, :],
                            in1=xt[m][:mw, tsl],
                            op0=mybir.AluOpType.add,
                            op1=mybir.AluOpType.add,
                        )
                        nc.sync.dma_start(
                            out=xout[b, MS[m] : MS[m] + mw, tsl], in_=o_sb[:mw, m, :]
                        )

    nc.finalize()
    return nc


def _prep_weights(Wq, Wk, Wv, Wo, bo):
    def pad_cols(W):
        # [cin, 320] -> [cin, 512] with col 64h+d = W[:, 40h+d]
        out = np.zeros((W.shape[0], CP), np.float32)
        for h in range(HEADS):
            out[:, DP * h : DP * h + DH] = W[:, DH * h : DH * h + DH]
        return out

    def chunk_rows(W, nchunk):
        # [cin, X] -> [128, nchunk, X] zero-padded
        out = np.zeros((128, nchunk, W.shape[1]), np.float32)
        for kk in range(nchunk):
            rows = W[kk * 128 : (kk + 1) * 128]
            out[: rows.shape[0], kk, :] = rows
        return out

    wq_a = chunk_rows(pad_cols(Wq), 3)
    wk_a = chunk_rows(pad_cols(Wk), 6)
    wv_a = chunk_rows(pad_cols(Wv), 6)

    wo_pad = np.zeros((CP, C), np.float32)
    for h in range(HEADS):
        wo_pad[DP * h : DP * h + DH, :] = Wo[DH * h : DH * h + DH, :]
    wo_a = chunk_rows(wo_pad, 4)

    sel = np.zeros((8, 4, 128), np.float32)
    for c in range(4):
        sel[c, c, 0:64] = 1.0       # head 2c   (den8 row c)
        sel[c + 4, c, 64:128] = 1.0  # head 2c+1 (den8 row c+4)

    bo_a = bo.astype(np.float32).reshape(C, 1)
    ident = np.eye(128, dtype=np.float32)
    return dict(wq=wq_a, wk=wk_a, wv=wv_a, wo=wo_a, sel=sel, bo=bo_a, ident=ident)


class _Runner:
    """Persistent jitted SPMD runner (jit once, reuse across calls)."""

    def __init__(self):
        import jax
        from jax.experimental.shard_map import shard_map
        from jax.sharding import Mesh, NamedSharding, PartitionSpec

        from concourse import bass2jax
        from concourse.bass2jax import _bass_exec_p, partition_id_tensor

        bass2jax.install_neuronx_cc_hook()
        nc = _build_nc()
        self.jax = jax

        partition_name = (
            nc.partition_id_tensor.name if nc.partition_id_tensor else None
        )
        in_names, out_names, out_avals = [], [], []
        for alloc in nc.m.functions[0].allocations:
            if not isinstance(alloc, mybir.MemoryLocationSet):
                continue
            name = alloc.memorylocations[0].name
            if alloc.kind == "ExternalInput":
                if name != partition_name:
                    in_names.append(name)
            elif alloc.kind == "ExternalOutput":
                out_names.append(name)
                out_avals.append(
                    jax.core.ShapedArray(
                        tuple(alloc.tensor_shape), mybir.dt.np(alloc.dtype)
                    )
                )
        n_params = len(in_names)
        all_names = in_names + out_names
        if partition_name is not None:
            all_names.append(partition_name)
        self.in_names = in_names
        self.out_names = out_names
        self.out_avals = out_avals

        def _body(*args):
            operands = list(args)
            if partition_name is not None:
                operands.append(partition_id_tensor())
            outs = _bass_exec_p.bind(
                *operands,
                out_avals=tuple(out_avals),
                in_names=tuple(all_names),
                out_names=tuple(out_names),
                lowering_input_output_aliases=(),
                sim_require_finite=True,
                sim_require_nnan=True,
                nc=nc,
            )
            return tuple(outs)

        self.body = _body
        devices = jax.devices()[:NCORES]
        self.mesh = Mesh(np.asarray(devices), ("core",))
        self.sharding = NamedSharding(self.mesh, PartitionSpec("core"))
        n_outs = len(out_names)
        in_specs = (PartitionSpec("core"),) * (n_params + n_outs)
        out_specs = (PartitionSpec("core"),) * n_outs
        self.fn = jax.jit(
            shard_map(
                _body,
                mesh=self.mesh,
                in_specs=in_specs,
                out_specs=out_specs,
                check_rep=False,
            ),
            keep_unused=True,
        )
        self.zero_outs = [
            np.zeros((NCORES * a.shape[0], *a.shape[1:]), a.dtype) for a in out_avals
        ]

    def pack(self, in_maps):
        """in_maps: list of per-core dicts -> list of concatenated global arrays."""
        concat = [
            np.concatenate([np.asarray(m[name]) for m in in_maps], axis=0)
            for name in self.in_names
        ]
        return concat + list(self.zero_outs)

    def device_put(self, args):
        return [self.jax.device_put(a, self.sharding) for a in args]

    def run(self, args):
        outs = self.fn(*args)
        return [np.asarray(o) for o in outs]


def _get_runner():
    if "runner" not in _CACHE:
        _CACHE["runner"] = _Runner()
    return _CACHE["runner"]


def _make_in_maps(hidden_states, encoder_hidden_states, Wq, Wk, Wv, Wo, bo):
    hidden_states = np.asarray(hidden_states, np.float32)
    encoder_hidden_states = np.asarray(encoder_hidden_states, np.float32)
    B = hidden_states.shape[0]
    wmap = _prep_weights(
        np.asarray(Wq, np.float32),
        np.asarray(Wk, np.float32),
        np.asarray(Wv, np.float32),
        np.asarray(Wo, np.float32),
        np.asarray(bo, np.float32),
    )
    x = hidden_states.reshape(B, C, HW)
    enc = encoder_hidden_states[:, :SK, :]
    in_maps = []
    for i in range(NCORES):
        m = dict(wmap)
        m["xin"] = np.ascontiguousarray(x[BPC * i : BPC * (i + 1)])
        m["enc"] = np.ascontiguousarray(enc[BPC * i : BPC * (i + 1)])
        in_maps.append(m)
    return in_maps


def kernel(hidden_states, encoder_hidden_states, Wq, Wk, Wv, Wo, bo):
    B, Cc, Hh, Ww = np.asarray(hidden_states).shape
    assert (B, Cc, Hh * Ww) == (16, C, HW)
    runner = _get_runner()
    in_maps = _make_in_maps(
        hidden_states, encoder_hidden_states, Wq, Wk, Wv, Wo, bo
    )
    outs = runner.run(runner.pack(in_maps))
    out = outs[runner.out_names.index("xout")]
    out = out.reshape(NCORES, BPC, C, HW).reshape(B, C, Hh, Ww)
    return np.ascontiguousarray(out.astype(np.float32))


# revision 9
# speedup vs baseline: 198.2917x; 198.2917x over previous
"""CNAttnProcessor cross-attention kernel for 8 Trainium2 NeuronCores.

Data-parallel over batch: 16 batches -> 2 per core. Per batch computes
  x = hidden[b] viewed [C=320, HW=4096]  (feature-major = "s-major")
  q^T = Wq_pad^T-arranged @ x           [512pad, 4096]
  k^T, v from enc[b,:77]                 (enc transposed on PE)
  scores^T_h = k_h^T.T @ q_h^T           [77, t]
  E = exp(SCALE * scores)                (unnormalized probs)
  attn_un^T = [v_h | 1col].T @ E_h       -> packed [512pad, t], den rows at 64h+63
  normalize via R = sel^T @ recip(den8)  (replication matmul, K=8)
  final^T = Wo_pad^T @ attn_norm^T + bo + residual
"""

import numpy as np

import concourse.bass as bass
import concourse.mybir as mybir
import concourse.tile as tile
from concourse import bacc, bass_utils

F32 = mybir.dt.float32
HEADS = 8
DH = 40          # real head dim
DP = 64          # padded head dim
C = 320
CP = 512         # padded channels (8 * 64)
SK = 77          # encoder tokens kept
DC = 768
HW = 4096
BPC = 2          # batches per core
NCORES = 8
TCH = 512        # token chunk
NT = HW // TCH   # 8 chunks
SCALE = float(1.0 / np.sqrt(np.float32(DH)))

_CACHE = {}


def _build_nc(iters=1):
    nc = bacc.Bacc("TRN2", target_bir_lowering=False, debug=False, num_devices=1)

    xin = nc.dram_tensor("xin", [BPC, C, HW], F32, kind="ExternalInput").ap()
    enc = nc.dram_tensor("enc", [BPC, SK, DC], F32, kind="ExternalInput").ap()
    wq = nc.dram_tensor("wq", [128, 3, CP], F32, kind="ExternalInput").ap()
    wk = nc.dram_tensor("wk", [128, 6, CP], F32, kind="ExternalInput").ap()
    wv = nc.dram_tensor("wv", [128, 6, CP], F32, kind="ExternalInput").ap()
    wo = nc.dram_tensor("wo", [128, 4, C], F32, kind="ExternalInput").ap()
    sel = nc.dram_tensor("sel", [8, 4, 128], F32, kind="ExternalInput").ap()
    bo = nc.dram_tensor("bo", [C, 1], F32, kind="ExternalInput").ap()
    ident = nc.dram_tensor("ident", [128, 128], F32, kind="ExternalInput").ap()
    xout = nc.dram_tensor("xout", [BPC, C, HW], F32, kind="ExternalOutput").ap()

    MW = [128, 128, 64]          # C chunks (320)
    MS = [0, 128, 256]
    KWQ = [128, 128, 64]         # contraction chunks for C=320

    with tile.TileContext(nc) as tc:
        with (
            tc.tile_pool(name="const", bufs=1) as const,
            tc.tile_pool(name="xpool", bufs=1) as xpool,
            tc.tile_pool(name="small", bufs=2) as small,
            tc.tile_pool(name="qpool", bufs=2) as qpool,
            tc.tile_pool(name="epool", bufs=2) as epool,
            tc.tile_pool(name="apool", bufs=2) as apool,
            tc.tile_pool(name="opool", bufs=2) as opool,
            tc.tile_pool(name="dpool", bufs=3) as dpool,
            tc.tile_pool(name="ps", bufs=2, space="PSUM") as ps,
        ):
            # ---- constants (once) ----
            wq_sb = const.tile([128, 3, CP], F32)
            nc.sync.dma_start(out=wq_sb, in_=wq)
            wk_sb = const.tile([128, 6, CP], F32)
            nc.sync.dma_start(out=wk_sb, in_=wk)
            wv_sb = const.tile([128, 6, CP], F32)
            nc.sync.dma_start(out=wv_sb, in_=wv)
            wo_sb = const.tile([128, 4, C], F32)
            nc.sync.dma_start(out=wo_sb, in_=wo)
            sel_sb = const.tile([8, 4, 128], F32)
            nc.sync.dma_start(out=sel_sb, in_=sel)
            bo_sb = const.tile([128, 3, 1], F32)
            for m in range(3):
                nc.sync.dma_start(out=bo_sb[: MW[m], m, :], in_=bo[MS[m] : MS[m] + MW[m], :])
            id_sb = const.tile([128, 128], F32)
            nc.sync.dma_start(out=id_sb, in_=ident)

            for b in [bb for _ in range(iters) for bb in range(BPC)]:
                # ---- load x^T (feature-major; also the residual) ----
                xt = []
                for m in range(3):
                    t = xpool.tile([128, HW], F32, tag=f"xt{m}")
                    nc.sync.dma_start(out=t[: MW[m], :], in_=xin[b, MS[m] : MS[m] + MW[m], :])
                    xt.append(t)

                # ---- enc -> enc^T via PE transpose ----
                enc_sb = small.tile([SK, DC], F32, tag="enc")
                nc.sync.dma_start(out=enc_sb, in_=enc[b])
                encT = small.tile([128, 6, SK], F32, tag="encT")
                for j in range(6):
                    ps_tr = ps.tile([128, 4, TCH], F32, tag="ps", name="ps_tr")[:, 0, :SK]
                    nc.tensor.transpose(
                        ps_tr[:, :SK], enc_sb[:, j * 128 : (j + 1) * 128], id_sb[:SK, :SK]
                    )
                    nc.any.tensor_copy(out=encT[:, j, :], in_=ps_tr)

                # ---- k^T (padded [512, 77] as 4 tiles) ----
                kt = []
                for m in range(4):
                    kps = ps.tile([128, 4, TCH], F32, tag="ps", name="kps")[:, 0, :SK]
                    for kk in range(6):
                        nc.tensor.matmul(
                            kps,
                            lhsT=wk_sb[:, kk, m * 128 : (m + 1) * 128],
                            rhs=encT[:, kk, :],
                            start=(kk == 0),
                            stop=(kk == 5),
                        )
                    t = small.tile([128, SK], F32, tag=f"kt{m}")
                    nc.any.tensor_copy(out=t, in_=kps)
                    kt.append(t)

                # ---- v' [77, 8, 64] with ones column at d=63 ----
                vps = ps.tile([128, 4, TCH], F32, tag="ps", name="vps")[:SK, 0, :CP]
                for kk in range(6):
                    nc.tensor.matmul(
                        vps,
                        lhsT=encT[:, kk, :],
                        rhs=wv_sb[:, kk, :],
                        start=(kk == 0),
                        stop=(kk == 5),
                    )
                v_sb = small.tile([SK, HEADS, DP], F32, tag="v")
                nc.any.tensor_copy(out=v_sb, in_=vps.rearrange("p (h d) -> p h d", h=HEADS))
                nc.vector.memset(v_sb[:, :, 63:64], 1.0)

                # ---- main loop over token chunks ----
                for n in range(NT):
                    tsl = bass.ts(n, TCH)

                    # Q projection for this chunk -> q_ps [128, 4, 512]
                    q_ps = ps.tile([128, 4, TCH], F32, tag="ps")
                    for m in range(4):
                        for kk in range(3):
                            kw = KWQ[kk]
                            nc.tensor.matmul(
                                q_ps[:, m, :],
                                lhsT=wq_sb[:kw, kk, m * 128 : (m + 1) * 128],
                                rhs=xt[kk][:kw, tsl],
                                start=(kk == 0),
                                stop=(kk == 2),
                            )
                    q_sb = qpool.tile([128, 4, TCH], F32, tag="q")
                    nc.any.tensor_copy(out=q_sb, in_=q_ps)

                    # scores + exp, two groups of 4 heads
                    e_sb = epool.tile([SK, HEADS, TCH], F32, tag="E")
                    for g in range(2):
                        sc_ps = ps.tile([128, 4, TCH], F32, tag="ps")
                        for j in range(4):
                            h = 4 * g + j
                            c, eps = h // 2, h % 2
                            nc.tensor.matmul(
                                sc_ps[:SK, j, :],
                                lhsT=kt[c][64 * eps : 64 * eps + DH, :],
                                rhs=q_sb[64 * eps : 64 * eps + DH, c, :],
                                start=True,
                                stop=True,
                            )
                        nc.scalar.activation(
                            out=e_sb[:SK, 4 * g : 4 * g + 4, :],
                            in_=sc_ps[:SK, :, :],
                            func=mybir.ActivationFunctionType.Exp,
                            scale=SCALE,
                        )

                    # AV (unnormalized) -> av_ps [128, 4, 512]; den rows at 64e+63
                    av_ps = ps.tile([128, 4, TCH], F32, tag="ps")
                    for h in range(HEADS):
                        c, eps = h // 2, h % 2
                        nc.tensor.matmul(
                            av_ps[64 * eps : 64 * eps + DP, c, :],
                            lhsT=v_sb[:SK, h, :],
                            rhs=e_sb[:SK, h, :],
                            start=True,
                            stop=True,
                        )

                    # unnormalized attn to SBUF (mandatory PSUM drain)
                    attn_un = qpool.tile([128, 4, TCH], F32, tag="r")
                    nc.any.tensor_copy(out=attn_un, in_=av_ps)

                    # gather denominators -> den8 [8, 512]; recip
                    den8 = dpool.tile([8, TCH], F32, tag="den")
                    for c in range(4):
                        nc.sync.dma_start(
                            out=den8[c : c + 5 : 4, :],
                            in_=attn_un[63 : 63 + 65 : 64, c, :],
                        )
                    rec8 = dpool.tile([8, TCH], F32, tag="rec")
                    nc.vector.reciprocal(out=rec8, in_=den8)

                    # replication matmul: R[:, c, :] rows 64e+j = recip(den of head 2c+e)
                    r_ps = ps.tile([128, 4, TCH], F32, tag="ps")
                    for c in range(4):
                        nc.tensor.matmul(
                            r_ps[:, c, :], lhsT=sel_sb[:, c, :], rhs=rec8, start=True, stop=True
                        )

                    # normalize: attn = attn_un * R   (R read from PSUM)
                    attn = apool.tile([128, 4, TCH], F32, tag="attn")
                    nc.vector.tensor_tensor(
                        out=attn, in0=attn_un, in1=r_ps, op=mybir.AluOpType.mult
                    )

                    # Wo projection (s-major out) + bias + residual
                    fo_ps = ps.tile([128, 3, TCH], F32, tag="ps")
                    for m in range(3):
                        mw = MW[m]
                        for kk in range(4):
                            nc.tensor.matmul(
                                fo_ps[:mw, m, :],
                                lhsT=wo_sb[:, kk, MS[m] : MS[m] + mw],
                                rhs=attn[:, kk, :],
                                start=(kk == 0),
                                stop=(kk == 3),
                            )
                    o_sb = opool.tile([128, 3, TCH], F32, tag="o")
                    for m in range(3):
                        mw = MW[m]
                        nc.vector.scalar_tensor_tensor(
                            out=o_sb[:mw, m, :],
                            in0=fo_ps[:mw, m, :],
                            scalar=bo_sb[:mw, m, :],
                            in1=xt[m][:mw, tsl],
                            op0=mybir.AluOpType.add,
                            op1=mybir.AluOpType.add,
                        )
                        nc.sync.dma_start(
                            out=xout[b, MS[m] : MS[m] + mw, tsl], in_=o_sb[:mw, m, :]
                        )

    nc.finalize()
    return nc


def _prep_weights(Wq, Wk, Wv, Wo, bo):
    def pad_cols(W):
        # [cin, 320] -> [cin, 512] with col 64h+d = W[:, 40h+d]
        out = np.zeros((W.shape[0], CP), np.float32)
        for h in range(HEADS):
            out[:, DP * h : DP * h + DH] = W[:, DH * h : DH * h + DH]
        return out

    def chunk_rows(W, nchunk):
        # [cin, X] -> [128, nchunk, X] zero-padded
        out = np.zeros((128, nchunk, W.shape[1]), np.float32)
        for kk in range(nchunk):
            rows = W[kk * 128 : (kk + 1) * 128]
            out[: rows.shape[0], kk, :] = rows
        return out

    wq_a = chunk_rows(pad_cols(Wq), 3)
    wk_a = chunk_rows(pad_cols(Wk), 6)
    wv_a = chunk_rows(pad_cols(Wv), 6)

    wo_pad = np.zeros((CP, C), np.float32)
    for h in range(HEADS):
        wo_pad[DP * h : DP * h + DH, :] = Wo[DH * h : DH * h + DH, :]
    wo_a = chunk_rows(wo_pad, 4)

    sel = np.zeros((8, 4, 128), np.float32)
    for c in range(4):
        sel[c, c, 0:64] = 1.0       # head 2c   (den8 row c)
        sel[c + 4, c, 64:128] = 1.0  # head 2c+1 (den8 row c+4)

    bo_a = bo.astype(np.float32).reshape(C, 1)
    ident = np.eye(128, dtype=np.float32)
    return dict(wq=wq_a, wk=wk_a, wv=wv_a, wo=wo_a, sel=sel, bo=bo_a, ident=ident)


class _Runner:
    """Persistent jitted SPMD runner (jit once, reuse across calls)."""

    def __init__(self, iters=1):
        import jax
        from jax.experimental.shard_map import shard_map
        from jax.sharding import Mesh, NamedSharding, PartitionSpec

        from concourse import bass2jax
        from concourse.bass2jax import _bass_exec_p, partition_id_tensor

        bass2jax.install_neuronx_cc_hook()
        nc = _build_nc(iters)
        self.jax = jax

        partition_name = (
            nc.partition_id_tensor.name if nc.partition_id_tensor else None
        )
        in_names, out_names, out_avals = [], [], []
        for alloc in nc.m.functions[0].allocations:
            if not isinstance(alloc, mybir.MemoryLocationSet):
                continue
            name = alloc.memorylocations[0].name
            if alloc.kind == "ExternalInput":
                if name != partition_name:
                    in_names.append(name)
            elif alloc.kind == "ExternalOutput":
                out_names.append(name)
                out_avals.append(
                    jax.core.ShapedArray(
                        tuple(alloc.tensor_shape), mybir.dt.np(alloc.dtype)
                    )
                )
        n_params = len(in_names)
        all_names = in_names + out_names
        if partition_name is not None:
            all_names.append(partition_name)
        self.in_names = in_names
        self.out_names = out_names
        self.out_avals = out_avals

        def _body(*args):
            operands = list(args)
            if partition_name is not None:
                operands.append(partition_id_tensor())
            outs = _bass_exec_p.bind(
                *operands,
                out_avals=tuple(out_avals),
                in_names=tuple(all_names),
                out_names=tuple(out_names),
                lowering_input_output_aliases=(),
                sim_require_finite=True,
                sim_require_nnan=True,
                nc=nc,
            )
            return tuple(outs)

        self.body = _body
        devices = jax.devices()[:NCORES]
        self.mesh = Mesh(np.asarray(devices), ("core",))
        self.sharding = NamedSharding(self.mesh, PartitionSpec("core"))
        n_outs = len(out_names)
        in_specs = (PartitionSpec("core"),) * (n_params + n_outs)
        out_specs = (PartitionSpec("core"),) * n_outs
        self.fn = jax.jit(
            shard_map(
                _body,
                mesh=self.mesh,
                in_specs=in_specs,
                out_specs=out_specs,
                check_rep=False,
            ),
            keep_unused=True,
        )
        self.zero_outs = [
            np.zeros((NCORES * a.shape[0], *a.shape[1:]), a.dtype) for a in out_avals
        ]

    def pack(self, in_maps):
        """in_maps: list of per-core dicts -> list of concatenated global arrays."""
        concat = [
            np.concatenate([np.asarray(m[name]) for m in in_maps], axis=0)
            for name in self.in_names
        ]
        return concat + list(self.zero_outs)

    def device_put(self, args):
        return [self.jax.device_put(a, self.sharding) for a in args]

    def run(self, args):
        outs = self.fn(*args)
        return [np.asarray(o) for o in outs]


def _get_runner():
    if "runner" not in _CACHE:
        _CACHE["runner"] = _Runner()
    return _CACHE["runner"]


def _make_in_maps(hidden_states, encoder_hidden_states, Wq, Wk, Wv, Wo, bo):
    hidden_states = np.asarray(hidden_states, np.float32)
    encoder_hidden_states = np.asarray(encoder_hidden_states, np.float32)
    B = hidden_states.shape[0]
    wmap = _prep_weights(
        np.asarray(Wq, np.float32),
        np.asarray(Wk, np.float32),
        np.asarray(Wv, np.float32),
        np.asarray(Wo, np.float32),
        np.asarray(bo, np.float32),
    )
    x = hidden_states.reshape(B, C, HW)
    enc = encoder_hidden_states[:, :SK, :]
    in_maps = []
    for i in range(NCORES):
        m = dict(wmap)
        m["xin"] = np.ascontiguousarray(x[BPC * i : BPC * (i + 1)])
        m["enc"] = np.ascontiguousarray(enc[BPC * i : BPC * (i + 1)])
        in_maps.append(m)
    return in_maps


def kernel(hidden_states, encoder_hidden_states, Wq, Wk, Wv, Wo, bo):
    B, Cc, Hh, Ww = np.asarray(hidden_states).shape
    assert (B, Cc, Hh * Ww) == (16, C, HW)
    runner = _get_runner()
    in_maps = _make_in_maps(
        hidden_states, encoder_hidden_states, Wq, Wk, Wv, Wo, bo
    )
    outs = runner.run(runner.pack(in_maps))
    out = outs[runner.out_names.index("xout")]
    out = out.reshape(NCORES, BPC, C, HW).reshape(B, C, Hh, Ww)
    return np.ascontiguousarray(out.astype(np.float32))
